# revision 1
# baseline (speedup 1.0000x reference)
"""MessagePassingConvolution kernel for 8 Trainium2 NeuronCores.

Strategy (all-bf16, PE-side replication, 2-deep software pipeline):
  - Host: sort edges by receiver; shard by receiver windows. Core m owns
    nodes [m*1280, (m+1)*1280) = 10 windows of 128 nodes. Each window's
    edge list is padded to a fixed budget (2176 = 17 subtiles of 128) so
    the SPMD program is identical across cores.
  - The per-edge einsum u[lo,e] = sum_ki Wg[ki,lo] h3[k,e] x[i,e] uses
    the ki -> (group, partition) split k = 32g + p//4, i = 4cx + p%4:
    only 2 distinct h3-replication patterns (one cheap PE matmul each,
    hb_g[p,e] = h3[32g+p//4,e] via constant 0/1 matrices) and 8 distinct
    x-replication tables, which the host precomputes and streams as
    plain bf16 DMA inputs (xs, tile-block-major, split over the sync and
    scalar queues). Act evacuates each hb group to SBUF bf16 so the DVE
    Hadamard A_g = hb_g * xs runs in 2x 16-bit all-SBUF mode; the PE
    then accumulates u[96,T] += Wg_c.T @ A over the 16 chunks.
  - One flat window-agnostic stream of 43 uniform tiles (4 subtiles of
    128 edges each; the last tile carries 2) over the 170-subtile core
    workload, with a two-iteration software pipeline: iteration gt runs
    the einsum of
    tile gt, the replication+Hadamard of tile gt+1, and the MLP of tile
    gt+2, plus lagged PE transposes (gt-1) and scatter matmuls (gt-2)
    as filler, so every PE instruction's inputs are ready ~a full
    iteration early and the tensor engine never stalls or down-clocks.
  - Output side: PE transposes u to edge-major, DVE multiplies by the
    l-segmented edge_attrs (msgs), and the scatter accumulates
    psum_acc[128,288] += S_st.T @ msgs_st across a window's 17 subtiles
    (a tile's subtiles may span two windows; ut/msgs/scatter split per
    window segment), with all 17 S masks built in one DVE is_equal op
    per window.
    Input loads ride the sync queue, output stores the scalar queue, so
    stores never head-of-line-block loads.
  - Output: per-core [1280, 288] slices -> concat -> [10000, 32, 9].
"""

import sys
import numpy as np
from contextlib import ExitStack

sys.path.insert(0, "/opt/trn_rl_repo")

import concourse.bass as bass  # noqa: E402
import concourse.bacc as bacc  # noqa: E402
import concourse.mybir as mybir  # noqa: E402
import concourse.tile as tile  # noqa: E402
from concourse.bass_utils import run_bass_kernel_spmd  # noqa: E402

import ml_dtypes  # noqa: E402

BF16 = ml_dtypes.bfloat16

# ---- problem constants (hardcoded per spec) ----
N_NODES = 10000
N_EDGES = 160000
C = 32
RADIAL = 8
HID = 64
NL = 3
L_DIMS = (1, 3, 5)
NSH = 9  # sum(L_DIMS)
AVG_NUM_NEIGHBORS = 16.0

N_CORES = 8
WIN = 128                      # nodes per window (psum partitions)
WINS_PER_CORE = 10
NODES_PER_CORE = WIN * WINS_PER_CORE     # 1280
SUB = 128                      # edges per subtile
SUBS_PER_WIN = 17              # window edge budget = 2176 (data max 2155)
WIN_E = SUB * SUBS_PER_WIN     # 2176
E_CORE = WIN_E * WINS_PER_CORE  # 21760
N_SUBTILES = WINS_PER_CORE * SUBS_PER_WIN  # 170 subtiles, window-agnostic
N_TILES = (N_SUBTILES + 3) // 4            # 43 tiles (42x512 + 1x256)
N_CHUNK = 16                   # ki chunks (2048 / 128)
KA = 32                        # k-rows per chunk (A)
IB = 4                         # i-values per chunk (B); KA*IB = 128
NGRP = HID // KA               # 2 distinct h3-replication patterns
NXT = C // IB                  # 8 distinct x tables (host-built)
LO = NL * C                    # 96
F_OUT = NSH * C                # 288

FP32 = mybir.dt.float32
BF16_DT = mybir.dt.bfloat16

_CACHED = {}

ACT_FUNC = mybir.ActivationFunctionType.Silu


def _build_nc():
    nc = bacc.Bacc()

    ef = nc.dram_tensor("ef", [RADIAL, E_CORE], BF16_DT, kind="ExternalInput")
    xs = nc.dram_tensor("xs", [128, NXT * E_CORE], BF16_DT, kind="ExternalInput")
    at = nc.dram_tensor("at", [SUB, WINS_PER_CORE * SUBS_PER_WIN * NSH], BF16_DT,
                        kind="ExternalInput")
    rl = nc.dram_tensor("rl", [SUB, WINS_PER_CORE * SUBS_PER_WIN], BF16_DT,
                        kind="ExternalInput")
    w1 = nc.dram_tensor("w1", [RADIAL, HID], BF16_DT, kind="ExternalInput")
    w2 = nc.dram_tensor("w2", [HID, HID], BF16_DT, kind="ExternalInput")
    w3 = nc.dram_tensor("w3", [HID, HID], BF16_DT, kind="ExternalInput")
    wg = nc.dram_tensor("wg", [128, N_CHUNK * LO], BF16_DT, kind="ExternalInput")
    rb = nc.dram_tensor("rb", [HID, NGRP * 128], BF16_DT, kind="ExternalInput")
    iota = nc.dram_tensor("iota", [128, 128], BF16_DT, kind="ExternalInput")
    ident = nc.dram_tensor("ident", [128, 128], BF16_DT, kind="ExternalInput")
    out = nc.dram_tensor("out", [NODES_PER_CORE, F_OUT], FP32, kind="ExternalOutput")

    n_tiles = N_TILES  # 43

    def tile_info(gt):
        st0 = 4 * gt                               # first global subtile
        nsub = min(4, N_SUBTILES - st0)
        return st0, nsub, nsub * SUB

    with tile.TileContext(nc) as tc, ExitStack() as ctx:
        const_p = ctx.enter_context(tc.tile_pool(name="const", bufs=1))
        stream_p = ctx.enter_context(tc.tile_pool(name="stream", bufs=4))
        win_p = ctx.enter_context(tc.tile_pool(name="win", bufs=3))
        chunk_p = ctx.enter_context(tc.tile_pool(name="chunk", bufs=3))
        psum_mlp = ctx.enter_context(tc.tile_pool(name="pmlp", bufs=1, space="PSUM"))
        psum_hb = ctx.enter_context(tc.tile_pool(name="phb", bufs=3, space="PSUM"))
        psum_u = ctx.enter_context(tc.tile_pool(name="pu", bufs=2, space="PSUM"))
        psum_ut = ctx.enter_context(tc.tile_pool(name="put", bufs=1, space="PSUM"))
        psum_acc = ctx.enter_context(tc.tile_pool(name="pacc", bufs=1, space="PSUM"))

        # ---- one-time constants into SBUF (small weights on the sync
        # queue first so the PE can start; bulk constants ride scalar) ----
        w1_sb = const_p.tile([RADIAL, HID], BF16_DT)
        nc.sync.dma_start(w1_sb[:], w1[:])
        w2_sb = const_p.tile([HID, HID], BF16_DT)
        nc.sync.dma_start(w2_sb[:], w2[:])
        w3_sb = const_p.tile([HID, HID], BF16_DT)
        nc.sync.dma_start(w3_sb[:], w3[:])
        iota_sb = const_p.tile([128, 128], BF16_DT)
        nc.sync.dma_start(iota_sb[:], iota[:])
        ident_sb = const_p.tile([128, 128], BF16_DT)
        nc.sync.dma_start(ident_sb[:], ident[:])
        rb_sb = const_p.tile([HID, NGRP * 128], BF16_DT)
        nc.scalar.dma_start(rb_sb[:], rb[:])
        wg_sb = const_p.tile([128, N_CHUNK * LO], BF16_DT)
        nc.scalar.dma_start(wg_sb[:], wg[:])

        # pipeline state
        wstate = {}   # w -> dict(at, rl, s_all, ut, msgs, acc)
        tstate = {}   # gt -> dict(ef, x, u_sb)
        lofs = (0, 1, 4)

        def start_window(w):
            at_sb = win_p.tile([SUB, SUBS_PER_WIN, NSH], BF16_DT, tag="at",
                               name=f"at_w{w}")
            nc.sync.dma_start(
                at_sb[:].rearrange("p s m -> p (s m)"),
                at[:, w * SUBS_PER_WIN * NSH:(w + 1) * SUBS_PER_WIN * NSH])
            rl_sb = win_p.tile([SUB, SUBS_PER_WIN], BF16_DT, tag="rl",
                               name=f"rl_w{w}")
            nc.sync.dma_start(
                rl_sb[:], rl[:, w * SUBS_PER_WIN:(w + 1) * SUBS_PER_WIN])
            ut_sb = win_p.tile([SUB, SUBS_PER_WIN, LO], BF16_DT, tag="ut",
                               name=f"ut_w{w}")
            msgs_sb = win_p.tile([SUB, SUBS_PER_WIN, F_OUT], BF16_DT, tag="msgs",
                                 name=f"msgs_w{w}")
            s_all = win_p.tile([SUB, SUBS_PER_WIN, WIN], BF16_DT, tag="sall",
                               name=f"sall_w{w}")
            # all 17 subtile scatter masks in one DVE op:
            # s_all[p, st, n] = (iota[p, n] == rl[p, st])
            nc.vector.tensor_tensor(
                out=s_all[:],
                in0=iota_sb[:, None, :].to_broadcast([SUB, SUBS_PER_WIN, WIN]),
                in1=rl_sb[:, :, None].to_broadcast([SUB, SUBS_PER_WIN, WIN]),
                op=mybir.AluOpType.is_equal)
            wstate[w] = dict(at=at_sb, rl=rl_sb, ut=ut_sb, msgs=msgs_sb,
                             s_all=s_all, acc=None)

        def prefetch(gt):
            """Issue ef + xs DMA for tile gt (called one tile early).

            xs is laid out tile-block-major on the host: tile gt's block is
            NXT*tsz contiguous columns starting at NXT*base, ordered
            [cX, j]. The halves ride different queues (sync / scalar)."""
            st0, nsub, tsz = tile_info(gt)
            base = st0 * SUB
            ef_sb = stream_p.tile([RADIAL, 512], BF16_DT, tag="ef",
                                  name=f"ef_{gt}")
            nc.sync.dma_start(ef_sb[:, :tsz], ef[:, base:base + tsz])
            x_sb = stream_p.tile([128, NXT, 512], BF16_DT, tag="x",
                                 name=f"x_{gt}")
            half = (NXT // 2) * tsz
            nc.sync.dma_start(
                x_sb[:, :NXT // 2, :tsz],
                xs[:, NXT * base:NXT * base + half])
            nc.scalar.dma_start(
                x_sb[:, NXT // 2:, :tsz],
                xs[:, NXT * base + half:NXT * base + 2 * half])
            tstate[gt] = dict(ef=ef_sb, x=x_sb)

        def win_segments(st0, nsub):
            """Split [st0, st0+nsub) into (window, local0, s0, cnt) runs."""
            segs = []
            s = st0
            while s < st0 + nsub:
                w = s // SUBS_PER_WIN
                l = s % SUBS_PER_WIN
                cnt = min(SUBS_PER_WIN - l, st0 + nsub - s)
                segs.append((w, l, s - st0, cnt))
                s += cnt
            return segs

        def do_transposes(gt):
            """PE transposes of tile gt's u_sb into ut_ps, Act evac to ut_sb
            (split per window segment when the tile spans a boundary)."""
            st0, nsub, tsz = tile_info(gt)
            st = tstate[gt]
            ut_ps = psum_ut.tile([128, 4, LO], BF16_DT, tag="utp",
                                 name=f"utp_{gt}")
            for s in range(nsub):
                nc.tensor.transpose(
                    out=ut_ps[:, s, :],
                    in_=st["u_sb"][:, s * SUB:(s + 1) * SUB],
                    identity=ident_sb[:LO, :LO])
            for w, l, o, cnt in win_segments(st0, nsub):
                ut_sb = wstate[w]["ut"]
                nc.scalar.copy(ut_sb[:, l:l + cnt, :], ut_ps[:, o:o + cnt, :])

        def do_msgs(gt):
            """DVE: msgs = uT * attrs for tile gt's subtiles, l-segmented,
            split per window segment."""
            st0, nsub, tsz = tile_info(gt)
            for w, l0, o, cnt in win_segments(st0, nsub):
                ut_sb = wstate[w]["ut"]
                at_sb = wstate[w]["at"]
                msgs_sb = wstate[w]["msgs"]
                for l in range(NL):
                    dim = L_DIMS[l]
                    u_ap = ut_sb[:, l0:l0 + cnt, None,
                                 l * C:(l + 1) * C].to_broadcast(
                        [SUB, cnt, dim, C])
                    a_ap = at_sb[:, l0:l0 + cnt, lofs[l]:lofs[l] + dim]
                    a_ap = a_ap[:, :, :, None].to_broadcast(
                        [SUB, cnt, dim, C])
                    nc.vector.tensor_tensor(
                        out=msgs_sb[:, l0:l0 + cnt,
                                    lofs[l] * C:(lofs[l] + dim) * C].rearrange(
                            "p s (m c) -> p s m c", c=C),
                        in0=u_ap, in1=a_ap, op=mybir.AluOpType.mult)

        def do_scatter(gt):
            """PE scatter matmuls for tile gt's subtiles into their window
            accs (a tile may span two windows)."""
            st0, nsub, tsz = tile_info(gt)
            for s in range(nsub):
                sg = st0 + s
                w = sg // SUBS_PER_WIN
                l = sg % SUBS_PER_WIN
                ws = wstate[w]
                if ws["acc"] is None:
                    ws["acc"] = psum_acc.tile([WIN, F_OUT], FP32, tag="acc",
                                              name=f"acc_w{w}")
                nc.tensor.matmul(out=ws["acc"][:], lhsT=ws["s_all"][:, l, :],
                                 rhs=ws["msgs"][:, l, :],
                                 start=(l == 0),
                                 stop=(l == SUBS_PER_WIN - 1),
                                 skip_group_check=True)
                if l == SUBS_PER_WIN - 1:
                    # window complete: evacuate + store (scalar queue, so
                    # the store never blocks loads on the sync queue)
                    out_sb = stream_p.tile([WIN, F_OUT], FP32, tag="osb",
                                           name=f"osb_w{w}")
                    nc.scalar.copy(out_sb[:], ws["acc"][:])
                    nc.scalar.dma_start(out[w * WIN:(w + 1) * WIN, :],
                                        out_sb[:])
                    wstate.pop(w)

        start_window(0)
        prefetch(0)
        prefetch(1)

        def do_mlp_layer(gt, layer):
            """One z-matmul + silu for tile gt; layer in (1, 2, 3)."""
            _, _, tsz_ = tile_info(gt)
            st = tstate[gt]
            src = {1: st["ef"], 2: st.get("h1"), 3: st.get("h2")}[layer]
            wsb = {1: w1_sb, 2: w2_sb, 3: w3_sb}[layer]
            z = psum_mlp.tile([HID, 512], FP32, tag="z", name=f"z{layer}_{gt}")
            nc.tensor.matmul(out=z[:, :tsz_], lhsT=wsb[:], rhs=src[:, :tsz_],
                             start=True, stop=True, skip_group_check=True)
            h = stream_p.tile([HID, 512], BF16_DT, tag=f"h{layer}",
                              name=f"h{layer}_{gt}")
            nc.scalar.activation(h[:, :tsz_], z[:, :tsz_], ACT_FUNC)
            st[f"h{layer}"] = h

        def emit_hb_all(gt):
            """PE replication + Act evac + DVE Hadamard for tile gt's four
            k-groups. Runs one iteration before tile gt's einsum so the
            a-tiles are long ready when the u-matmuls arrive."""
            _, _, tsz_ = tile_info(gt)
            st = tstate[gt]
            h3 = st["h3"]
            x_sb = st["x"]
            a_all = chunk_p.tile([128, NGRP, NXT, 512], BF16_DT, tag="a",
                                 name=f"a_{gt}", bufs=2)
            for g in range(NGRP):
                hb = psum_hb.tile([128, 512], FP32, tag="hb",
                                  name=f"hb_{gt}_{g}")
                nc.tensor.matmul(
                    out=hb[:, :tsz_],
                    lhsT=rb_sb[:, g * 128:(g + 1) * 128],
                    rhs=h3[:, :tsz_],
                    start=True, stop=True, skip_group_check=True)
                # Act evacuates to SBUF bf16 so the DVE Hadamard runs in
                # 2x 16-bit all-SBUF mode (PSUM reads would be 1x)
                hbs = chunk_p.tile([128, 512], BF16_DT, tag="hbs",
                                   name=f"hbs_{gt}_{g}")
                nc.scalar.copy(hbs[:, :tsz_], hb[:, :tsz_])
                nc.vector.tensor_tensor(
                    out=a_all[:, g, :, :tsz_],
                    in0=hbs[:, None, :tsz_].to_broadcast([128, NXT, tsz_]),
                    in1=x_sb[:, :, :tsz_],
                    op=mybir.AluOpType.mult)
            st["a"] = a_all

        # prologue: tiles 0/1 MLP + tile 0 replication run un-pipelined
        do_mlp_layer(0, 1)
        do_mlp_layer(0, 2)
        do_mlp_layer(0, 3)
        do_mlp_layer(1, 1)
        do_mlp_layer(1, 2)
        do_mlp_layer(1, 3)
        emit_hb_all(0)

        next_w = 1
        for gt in range(n_tiles):
            st0, nsub, tsz = tile_info(gt)
            st = tstate[gt]

            # prefetches: windows started once within 3 tiles of first use,
            # tile data 2 tiles ahead
            while next_w < WINS_PER_CORE and \
                    next_w * SUBS_PER_WIN < 4 * (gt + 4):
                start_window(next_w)
                next_w += 1
            if gt + 2 < n_tiles:
                prefetch(gt + 2)

            u_ps = psum_u.tile([LO, 512], FP32, tag="u", name=f"u_{gt}")
            a_all = st["a"]

            def emit_u(g):
                for cx in range(NXT):
                    c = g * NXT + cx
                    nc.tensor.matmul(
                        out=u_ps[:, :tsz],
                        lhsT=wg_sb[:, c * LO:(c + 1) * LO],
                        rhs=a_all[:, g, cx, :tsz],
                        start=(c == 0), stop=(c == N_CHUNK - 1),
                        skip_group_check=True)

            # next tile's replication/Hadamard chain kicks off first
            if gt + 1 < n_tiles:
                emit_hb_all(gt + 1)
            if gt + 2 < n_tiles:
                do_mlp_layer(gt + 2, 1)
            if gt >= 1:
                do_transposes(gt - 1)
            if gt + 2 < n_tiles:
                do_mlp_layer(gt + 2, 2)
            emit_u(0)
            if gt >= 2:
                do_scatter(gt - 2)
            if gt + 2 < n_tiles:
                do_mlp_layer(gt + 2, 3)
            emit_u(1)

            # evacuate u (Act) for next-tile transposes
            u_sb = stream_p.tile([LO, 512], BF16_DT, tag="usb", name=f"usb_{gt}")
            nc.scalar.copy(u_sb[:, :tsz], u_ps[:, :tsz])
            st["u_sb"] = u_sb

            # msgs of previous tile (DVE)
            if gt >= 1:
                do_msgs(gt - 1)
            if gt >= 2:
                tstate.pop(gt - 2)

        # drain pipeline
        do_transposes(n_tiles - 1)
        do_msgs(n_tiles - 1)
        do_scatter(n_tiles - 2)
        do_scatter(n_tiles - 1)

    nc.compile()
    return nc


def _host_prep(node_feats, edge_attrs, edge_feats, senders, receivers,
               W1, W2, W3, Wgen):
    """Sort/shard edges by receiver window, build per-core input maps."""
    senders = np.asarray(senders).astype(np.int64)
    receivers = np.asarray(receivers).astype(np.int64)
    node_feats = np.asarray(node_feats, dtype=np.float32)
    edge_attrs = np.asarray(edge_attrs, dtype=np.float32)
    edge_feats = np.asarray(edge_feats, dtype=np.float32)

    n_win_total = N_CORES * WINS_PER_CORE  # 80
    win_id = receivers // WIN
    order = np.argsort(win_id, kind="stable")
    counts = np.bincount(win_id, minlength=n_win_total)
    assert counts.max() <= WIN_E, f"window overflow: {counts.max()} > {WIN_E}"
    starts = np.zeros(n_win_total + 1, np.int64)
    np.cumsum(counts, out=starts[1:])

    # slot arrays (padded); padding edges: ef=0, attr=0 -> msgs contribution 0
    E_TOT = N_CORES * E_CORE
    ef_s = np.zeros((E_TOT, RADIAL), np.float32)
    at_s = np.zeros((E_TOT, NSH), np.float32)
    rl_s = np.zeros(E_TOT, np.float32)
    sd_s = np.zeros(E_TOT, np.int64)

    slot_base = np.arange(n_win_total) * WIN_E
    # positions for real edges
    within = np.arange(len(order)) - starts[win_id[order]]
    slots = slot_base[win_id[order]] + within
    ef_s[slots] = edge_feats[order]
    at_s[slots] = edge_attrs[order] * np.float32(1.0 / np.sqrt(AVG_NUM_NEIGHBORS))
    rl_s[slots] = (receivers[order] % WIN).astype(np.float32)
    sd_s[slots] = senders[order]

    # host-side gather base: x values per edge, bf16
    nf_b = node_feats.astype(BF16)

    # weights with fan-in scales folded (bf16)
    w1 = (W1 * (1.0 / np.sqrt(RADIAL))).astype(BF16)
    w2 = (W2 * (1.0 / np.sqrt(HID))).astype(BF16)
    w3 = (W3 * (1.0 / np.sqrt(HID))).astype(BF16)
    # chunk c = g*NXT + cx: wg[p, c*96+lo] =
    #   Wgen[KA*g + p//IB, l, o, IB*cx + p%IB] * 1/sqrt(HID*C)
    wgen = np.asarray(Wgen, dtype=np.float32) * np.float32(1.0 / np.sqrt(HID * C))
    p = np.arange(128)
    wg = np.zeros((N_CHUNK, 128, NL, C), np.float32)
    for g in range(NGRP):
        for cx in range(NXT):
            wg[g * NXT + cx] = wgen[KA * g + p // IB][
                p, :, :, IB * cx + p % IB].reshape(128, NL, C)
    # -> [128, 16*96]: chunk-major along free dim
    wg = wg.reshape(N_CHUNK, 128, LO).transpose(1, 0, 2).reshape(128, N_CHUNK * LO)
    wg = wg.astype(BF16)

    # replication matrices: rb[q, g*128 + p] = (q == KA*g + p//IB)
    rb = np.zeros((HID, NGRP, 128), np.float32)
    for g in range(NGRP):
        rb[KA * g + p // IB, g, p] = 1.0
    rb = rb.reshape(HID, NGRP * 128).astype(BF16)

    iota = np.broadcast_to(np.arange(128, dtype=np.float32), (128, 128)).astype(BF16)
    ident = np.eye(128, dtype=np.float32).astype(BF16)

    in_maps = []
    for m in range(N_CORES):
        sl = slice(m * E_CORE, (m + 1) * E_CORE)
        ef_c = ef_s[sl]      # [E_CORE, 8]
        at_c = at_s[sl]      # [E_CORE, 9]
        rl_c = rl_s[sl]
        sd_c = sd_s[sl]
        n_st = E_CORE // SUB  # 170
        x_c = nf_b[sd_c]                       # [E_CORE, 32] bf16
        # xs_all[p, cx, e] = x[IB*cx + p%IB, e]; tile-block-major layout:
        # tile gt's block = xs_all[:, :, base:base+tsz] flattened (cx, j)
        xg = x_c.T.reshape(NXT, IB, E_CORE)     # [cx, i_lo, e]
        xs_all = np.tile(xg, (1, 128 // IB, 1)).reshape(NXT, 128, E_CORE)
        xs_all = xs_all.transpose(1, 0, 2)      # [128, cx, e]
        blocks = []
        for gt in range(N_TILES):
            b0 = gt * 4 * SUB
            tsz_ = min(4 * SUB, E_CORE - b0)
            blocks.append(xs_all[:, :, b0:b0 + tsz_].reshape(128, -1))
        xs_c = np.ascontiguousarray(np.concatenate(blocks, axis=1))
        in_maps.append({
            "ef": np.ascontiguousarray(ef_c.T).astype(BF16),
            "xs": xs_c,
            "at": np.ascontiguousarray(
                at_c.reshape(n_st, SUB, NSH).transpose(1, 0, 2).reshape(
                    SUB, n_st * NSH)).astype(BF16),
            "rl": np.ascontiguousarray(
                rl_c.reshape(n_st, SUB).T).astype(BF16),
            "w1": w1, "w2": w2, "w3": w3, "wg": wg, "rb": rb,
            "iota": np.ascontiguousarray(iota), "ident": ident,
        })
    return in_maps


def kernel(node_feats, edge_attrs, edge_feats, senders, receivers,
           W1, W2, W3, Wgen):
    in_maps = _host_prep(node_feats, edge_attrs, edge_feats, senders, receivers,
                         W1, W2, W3, Wgen)
    if "nc" not in _CACHED:
        _CACHED["nc"] = _build_nc()
    nc = _CACHED["nc"]
    res = run_bass_kernel_spmd(nc, in_maps, core_ids=list(range(N_CORES)))
    outs = [res.results[m]["out"] for m in range(N_CORES)]
    full = np.concatenate(outs, axis=0)[:N_NODES]          # [10000, 288]
    out = full.reshape(N_NODES, NSH, C).transpose(0, 2, 1)  # [10000, 32, 9]
    return np.ascontiguousarray(out.astype(np.float32))

